# revision 25
# baseline (speedup 1.0000x reference)
"""Chamfer distance kernel for Trainium2 (8 NeuronCores, batch-parallel).

Problem: input1 (8,4096,3), input2 (8,4096,3) fp32.
  D[b,n,m] = ||input1[b,n]-input2[b,m]||
  loss = mean_b( mean_m min_n D + mean_n min_m D )

Per core (one batch): -D2 = 2*x1.x2 - n1[n] - n2[m] computed on the PE as a
single K=13 float32r matmul whose contraction rows carry the hi/lo split of
the coordinates plus the hi/lo split of both squared norms. The sign is
flipped so both reductions are MAX.

Design (v8):
- Staging is done ON THE HOST: kernel() builds the 13-row L/R contraction
  operands in numpy (hi = round-to-10-explicit-mantissa-bits, which the
  PE's TF32-like f32r operand precision preserves exactly - a 12-bit hi
  measured 15% loss error on HW from the PE re-rounding it; lo = x - hi
  exact by Sterbenz) and ships them as the DRAM inputs, column-chunked
  across both HWDGE queues so tile 0's operands land first. Device setup
  is just these DMAs + a colmax-accumulator memset + a dummy Sqrt (loads
  the sqrt_and_others act table - covers the loop's Copy and the tail's
  Relu/Sqrt - off the critical path) + 8 warmup matmuls on a zeroed
  scratch that ramp the PE out of its low p-state while the DMAs fly.
  (v5 staged on-device: ~19us of DVE math + scatter DMAs.)
- Main loop per 128-row I-tile: 8 matmuls fill two 2048-col PSUM groups
  G0/G1 (all 8 banks). PSUM egress is the hard wall: only ScalarE and
  VectorE can read PSUM (GPSIMD cannot, and DMA has no PSUM endpoint),
  matmul output must be fp32, and only bf16-SBUF tensor_tensor runs in the
  fast DVE perf mode - so ScalarE copies both groups into one contiguous
  bf16 tile C (~2.06us/tile) and DVE does column-max first (one full-width
  bf16 accumulate over a ping-pong pair; out != in0 keeps the fast mode),
  then rowmax via pairwise-max halving of C - 3 contiguous bf16
  tensor_tensors stopping at width 512 (~2.06us/tile). ScalarE and DVE are
  dead even at the measured 2.06us/tile loop period - the loop is at the
  egress/reduce roofline for this algorithm.
- Tail: batch-fold rg5 to one value per I-tile (9 block-strided packed
  TTs), gpsimd partition_all_reduce on the colmax accumulator in two
  pipelined halves, gather, clamp+sqrt on ACT, two tiny output DMAs; host
  means the [128, IT_N] outputs. In whole-kernel For_i replication the
  tail pipelines into the next iteration's loop and adds ~nothing to the
  steady-state period (measured 66.1us full-kernel slope vs 66.0us
  loop-only).
  REJECTED alternatives (all measured slower on HW):
  * streaming rg/colmax to DRAM for a host-side finish: the loop already
    saturates SBUF ports, so in-loop SBUF->DRAM DMAs slow it ~1:1
    (+17us loop, full slope 114.9us);
  * progressive mid-loop chunk folds: scheduling pathology, +100us;
  * tensor_tensor_reduce (fuse fold+reduce): hard-crashes the device;
  * GPSIMD tensor ops in the loop: fail BIR engine checks.
Host averages the per-core sums (the batch mean is the unshard step).
"""

import sys

sys.path.insert(0, "/opt/trn_rl_repo")

import numpy as np
from contextlib import ExitStack

import concourse.bacc as bacc
import concourse.tile as tile
import concourse.bass_isa as bass_isa
from concourse import mybir
from concourse.bass_utils import run_bass_kernel_spmd

B, NPTS, KDIM = 8, 4096, 3
IT_N = NPTS // 128   # 32 I-tiles of 128 rows (x1 points)
HALF = NPTS // 2     # 2048: cols per PSUM group

F32 = mybir.dt.float32
F32R = mybir.dt.float32r
BF16 = mybir.dt.bfloat16
KROWS = 13
RGW = 512  # in-loop rowfold halving stops here; tail batch-folds the rest

_cached = {}


def _rnd10(a: np.ndarray) -> np.ndarray:
    """Round fp32 to 10 explicit mantissa bits (round-half-up on magnitude).

    The PE's f32r operand precision is TF32-like: a 10-bit hi passes
    through the matmul unrounded, so lo = x - hi is exact and the hi/lo
    pair reconstructs x. The dropped lo1*lo2 cross term is ~2^-21*|x1||x2|,
    ~1% of the smallest D2 values, random sign, averages out in the mean.
    """
    u = np.ascontiguousarray(a, dtype=np.float32).view(np.uint32)
    r = (u + np.uint32(0x1000)) & np.uint32(0xFFFFE000)
    return r.view(np.float32)


def stage_host(x1: np.ndarray, x2: np.ndarray):
    """Build the [13, NPTS] f32 L/R contraction-row operands for one batch.

    sum_r L[r,n]*R[r,m] = 2*x1[n].x2[m] - |x1[n]|^2 - |x2[m]|^2 = -D2[n,m]
    (up to the dropped x1lo*ylo term).
    """
    x1 = np.ascontiguousarray(x1, dtype=np.float32)
    x2 = np.ascontiguousarray(x2, dtype=np.float32)
    y = (np.float32(2.0) * x2).astype(np.float32)
    x1h = _rnd10(x1)
    x1l = (x1 - x1h).astype(np.float32)
    yh = _rnd10(y)
    yl = (y - yh).astype(np.float32)
    n1 = (x1 * x1).sum(axis=1, dtype=np.float32)
    n1h = _rnd10(n1)
    n1l = (n1 - n1h).astype(np.float32)
    m2 = (-(x2 * x2).sum(axis=1, dtype=np.float32)).astype(np.float32)
    m2h = _rnd10(m2)
    m2l = (m2 - m2h).astype(np.float32)
    L = np.empty((KROWS, NPTS), np.float32)
    L[0:3] = x1h.T
    L[3:6] = x1h.T
    L[6:9] = x1l.T
    L[9] = n1h
    L[10] = n1l
    L[11] = 1.0
    L[12] = 1.0
    R = np.empty((KROWS, NPTS), np.float32)
    R[0:3] = yh.T
    R[3:6] = yl.T
    R[6:9] = yh.T
    R[9] = -1.0
    R[10] = -1.0
    R[11] = m2h
    R[12] = m2l
    return L, R


def finish_host(outc: np.ndarray, outr: np.ndarray) -> float:
    """Host-side mean of the per-point sqrt'd min distances for one core."""
    return float(
        np.asarray(outc).mean(dtype=np.float64)
        + np.asarray(outr).mean(dtype=np.float64)
    )


def _build(reps: int = 1, loop_n: int = 1, whole: bool = False, serial: bool = False,
           rgw: int = RGW, cbufs: int = 3):
    """whole=False: loop_n replicates only the main loop (For_i).
    whole=True: loop_n replicates the ENTIRE kernel body (setup + main
    loop + tail) inside one For_i; the wall-clock slope over loop_n
    measures the steady-state whole-kernel device time with the real
    (not cost-model) tail. serial=True additionally ends each iteration
    with an explicit all-engine barrier so iterations cannot pipeline:
    the slope is then the true cold single-shot latency."""
    nc = bacc.Bacc("TRN2", target_bir_lowering=False, debug=False, num_devices=B)

    L_d = nc.dram_tensor("L", [KROWS, NPTS], F32R, kind="ExternalInput").ap()
    R_d = nc.dram_tensor("R", [KROWS, NPTS], F32R, kind="ExternalInput").ap()
    outc_d = nc.dram_tensor("outc", [128, IT_N], F32, kind="ExternalOutput").ap()
    outr_d = nc.dram_tensor("outr", [128, IT_N], F32, kind="ExternalOutput").ap()

    MX = mybir.AluOpType.max

    import contextlib
    with tile.TileContext(nc) as tc, ExitStack() as ctx:
      sb = ctx.enter_context(tc.tile_pool(name="sb", bufs=1))
      cbp = ctx.enter_context(tc.tile_pool(name="cbp", bufs=cbufs))
      jkp = ctx.enter_context(tc.tile_pool(name="jkp", bufs=2))
      ps = ctx.enter_context(tc.tile_pool(name="ps", bufs=1, space="PSUM"))
      whole_ctx = tc.For_i(0, loop_n, 1) if (whole and loop_n > 1) else contextlib.nullcontext()
      with whole_ctx:
        L = sb.tile([KROWS, NPTS], F32R)
        R = sb.tile([KROWS, NPTS], F32R)

        # ---- setup ----
        # host-staged operands, column-chunked so tile 0's first slices
        # (L cols 0:128, R cols 0:512) land within ~1.5us and the loop
        # starts ~immediately; later chunks arrive under the early tiles
        nc.sync.dma_start(L[:, 0:512], L_d[:, 0:512])
        nc.scalar.dma_start(R[:, 0:512], R_d[:, 0:512])
        nc.sync.dma_start(L[:, 512:NPTS], L_d[:, 512:NPTS])
        nc.scalar.dma_start(R[:, 512:HALF], R_d[:, 512:HALF])
        nc.scalar.dma_start(R[:, HALF:NPTS], R_d[:, HALF:NPTS])

        # act-table preload: the first activation being a Sqrt makes the
        # framework load sqrt_and_others (also holds Copy/Relu/Identity)
        # here, overlapped with the DMAs, instead of mid-loop or mid-tail
        dm0 = sb.tile([1, 2], F32)
        dm1 = sb.tile([1, 2], F32)
        nc.gpsimd.memset(dm0[:], 4.0)
        nc.scalar.activation(dm1[:], dm0[:], mybir.ActivationFunctionType.Sqrt)

        # PE p-state warmup: a few matmuls on a zeroed scratch push the PE
        # through its low->mid clock ramp while the input DMAs are in
        # flight, so the first real I-tiles run near full clock. Kept to 4
        # so the warmup (PE queue is in-order) ends about when the first
        # input chunks land (~1.5us) and never delays tile 0.
        # (Scratch lhsT/rhs live in SBUF; output reuses the G1 PSUM space.)
        wz = sb.tile([KROWS, 640], F32R)
        nc.gpsimd.memset(wz[:].bitcast(F32), 0.0)
        WP = ps.tile([128, HALF], F32, tag="g1")
        for _ in range(4):
            nc.tensor.matmul(
                WP[:, 0:512], wz[:, 0:128], wz[:, 128:640],
                start=True, stop=True,
            )

        # ---- colmax accumulators (ping-pong keeps bf16 TT in fast mode) ----
        cmb_a = sb.tile([128, NPTS], BF16, tag="cma")
        cmb_b = sb.tile([128, NPTS], BF16, tag="cmb")
        nc.vector.memset(cmb_a[:], -3.0e38)
        rg5 = sb.tile([128, IT_N * rgw], BF16)
        rgf = sb.tile([128, IT_N], BF16)

        # ---- main loop ----
        # (reps/loop_n repeat the identical main loop for differential HW timing)
        loop_ctx = tc.For_i(0, loop_n, 1) if (loop_n > 1 and not whole) else contextlib.nullcontext()
        with loop_ctx:
          for _rep in range(reps):
            for it in range(IT_N):
                Ls = L[:, it * 128 : (it + 1) * 128]
                G0 = ps.tile([128, HALF], F32)
                for j in range(4):
                    nc.tensor.matmul(
                        G0[:, j * 512 : (j + 1) * 512], Ls,
                        R[:, j * 512 : (j + 1) * 512],
                        start=True, stop=True,
                    )
                C = cbp.tile([128, NPTS], BF16, tag="c")
                nc.scalar.copy(C[:, 0:HALF], G0[:])
                G1 = ps.tile([128, HALF], F32, tag="g1")
                for j in range(4):
                    nc.tensor.matmul(
                        G1[:, j * 512 : (j + 1) * 512], Ls,
                        R[:, HALF + j * 512 : HALF + (j + 1) * 512],
                        start=True, stop=True,
                    )
                nc.scalar.copy(C[:, HALF:NPTS], G1[:])
                # colmax first (one full-width bf16 accumulate): the tail's
                # partition_all_reduce only waits on the LAST colmax, so the
                # final I-tile's rowmax halving overlaps it
                src, dst = (cmb_a, cmb_b) if it % 2 == 0 else (cmb_b, cmb_a)
                nc.vector.tensor_tensor(dst[:], src[:], C[:], op=MX)
                # rowmax via pairwise-max halving (contiguous bf16
                # tensor_tensor stays in the fast mode; tensor_reduce would
                # be stuck at 1 elem/cycle). Stop at width 512: the
                # remaining fold levels run batched in the tail, where they
                # pipeline into the next iteration / overlap the gpsimd
                # partition reduce.
                w = NPTS // 2
                prev = C
                while w > rgw:
                    t = jkp.tile([128, w], BF16, tag=f"tr{w}")
                    nc.vector.tensor_tensor(
                        t[:], prev[:, 0:w], prev[:, w : 2 * w], op=MX
                    )
                    prev = t
                    w //= 2
                nc.vector.tensor_tensor(
                    rg5[:, it * rgw : (it + 1) * rgw],
                    prev[:, 0:rgw], prev[:, rgw : 2 * rgw], op=MX,
                )

        # ---- tail ----
        # batch-fold rg5 [128, 32*512] down to one value per I-tile with
        # block-strided packed TTs (the fast 1-port mode only constrains
        # the innermost dim)
        cur, cw = rg5[:], rgw
        while cw > 2:
            half = cw // 2
            # tail-only temporaries: bufs=1 (no cross-iteration overlap
            # needed; keeps SBUF within budget at larger rgw)
            nxt = jkp.tile([128, IT_N * half], BF16, tag=f"rgc{half}", bufs=1)
            cv = cur.rearrange("p (i w) -> p i w", w=cw)
            nc.vector.tensor_tensor(
                nxt[:].rearrange("p (i w) -> p i w", w=half),
                cv[:, :, 0:half], cv[:, :, half:cw], op=MX,
            )
            cur, cw = nxt[:], half
        cv = cur.rearrange("p (i w) -> p i w", w=2)
        nc.vector.tensor_tensor(
            rgf[:].rearrange("p (i w) -> p i w", w=1),
            cv[:, :, 0:1], cv[:, :, 1:2], op=MX,
        )

        cmb_fin = cmb_a if (IT_N * reps) % 2 == 0 else cmb_b
        o0 = sb.tile([128, IT_N], F32)
        cmr = sb.tile([128, NPTS], BF16)
        cmd = sb.tile([128, IT_N], BF16)
        # partition-reduce in two halves so the gather/clamp/sqrt of half 0
        # overlaps the reduce of half 1
        for h in range(2):
            hw_ = IT_N // 2
            nc.gpsimd.partition_all_reduce(
                cmr[:, h * HALF : (h + 1) * HALF],
                cmb_fin[:, h * HALF : (h + 1) * HALF],
                channels=128, reduce_op=bass_isa.ReduceOp.max,
            )
            nc.sync.dma_start(
                cmd[:, h * hw_ : (h + 1) * hw_],
                cmr[0:1, h * HALF : (h + 1) * HALF].rearrange(
                    "o (p t) -> o p t", p=128
                ),
            )
            # clamp+negate+sqrt entirely on ACT: sqrt(-min(x,0)) =
            # sqrt(Relu(-x)) - keeps DVE's tail free for the rowmax fold
            ngh = jkp.tile([128, IT_N // 2], F32, tag="ng")
            nc.scalar.activation(
                ngh[:], cmd[:, h * hw_ : (h + 1) * hw_],
                mybir.ActivationFunctionType.Relu, scale=-1.0,
            )
            nc.scalar.activation(
                o0[:, h * hw_ : (h + 1) * hw_], ngh[:],
                mybir.ActivationFunctionType.Sqrt,
            )
        o1 = sb.tile([128, IT_N], F32)
        ngr = sb.tile([128, IT_N], F32)
        nc.scalar.activation(
            ngr[:], rgf[:], mybir.ActivationFunctionType.Relu, scale=-1.0
        )
        nc.scalar.activation(o1[:], ngr[:], mybir.ActivationFunctionType.Sqrt)
        nc.sync.dma_start(outc_d[:], o0[:])
        nc.sync.dma_start(outr_d[:], o1[:])
        if whole and serial and loop_n > 1:
            nc.all_engine_barrier()

    nc.compile()
    return nc


def _get(reps: int = 1, loop_n: int = 1, **kw):
    key = (reps, loop_n, tuple(sorted(kw.items())))
    if key not in _cached:
        _cached[key] = _build(reps, loop_n, **kw)
    return _cached[key]


def kernel(input1: np.ndarray, input2: np.ndarray, _trace: bool = False):
    nc = _get()
    input1 = np.ascontiguousarray(np.asarray(input1, dtype=np.float32))
    input2 = np.ascontiguousarray(np.asarray(input2, dtype=np.float32))
    in_maps = []
    for b in range(B):
        Lb, Rb = stage_host(input1[b], input2[b])
        in_maps.append({"L": Lb, "R": Rb})
    res = run_bass_kernel_spmd(nc, in_maps, core_ids=list(range(B)), trace=_trace)
    losses = []
    for b in range(B):
        r = res.results[b]
        losses.append(finish_host(r["outc"], r["outr"]))
    out = np.float32(np.mean(losses))
    if _trace:
        return out, res
    return out
